# revision 9
# baseline (speedup 1.0000x reference)
"""MetaRoPE kernel for Trainium2, 8 NeuronCores — fp16 I/O, 2x-mode DVE,
merged muls, partial GpSimd offload.

Reference computation:
    r = rotate_m[token_positions]            # [S, D, D], block-diag 2x2 rotations
    out = einsum('bhsi,soi->bhso', x, r)     # x: [4, 32, 4096, 64] fp32

Because r is block-diagonal with 2x2 blocks, out = x * A + pairswap(x * B')
with host-precomputed tables A, B' of shape [S, D] (see _tables).

Precision/bandwidth: the harness gate is rel_err < 2e-2; fp16 end-to-end
(host converts x fp32->fp16, device computes in fp16, host converts the
fp16 result back) measures ~1.1e-3 and halves both HBM traffic and DVE
element cost. Plain InstTensorTensor ops hit the DVE 2x_1p perf mode with
packed fp16 (~0.49 ns/elem/partition measured, even with the stride -1
pair-swap operand); scalar_tensor_tensor would disable all perf modes.

Sharding: x reshaped to [128 (b,h) slabs, 4096, 64]; 16 slabs per core.
Each slab [4096*64] is viewed as [128 partitions, 2048 free] (contiguous per
partition; partition p holds positions 32p..32p+31). Tables are replicated
per core as one [128, 2*FREE] fp16 tile (tb | ta) matching that layout.

Per core the 16 slabs are processed in chunks (CHUNK_PLAN, tapered small at
the ends). Each chunk: one load (sync ring); ONE merged DVE multiply
computing u = x*tb and o = x*ta into one [128, 2*cfree] tile (x broadcast
via a step-0 AP dim, tables broadcast across slabs); one pair-swapped
in-place add o += pairswap(u) split by columns between DVE and GpSimd
(POOL_ADD_FRAC); one store (scalar ring). Steady state aims DMA-bound
(~17.9 MB/core at ~360 GB/s => ~50 us) with DVE (~38 us) + GpSimd (~16 us)
hidden underneath.
"""

import sys

import numpy as np

_TRN_REPO = "/opt/trn_rl_repo"
if _TRN_REPO not in sys.path:
    sys.path.insert(0, _TRN_REPO)

B, H, S, D = 4, 32, 4096, 64
BH = B * H                      # 128 (b,h) slabs
N_CORES = 8
BH_PER_CORE = BH // N_CORES     # 16 slabs per core
FREE = (S // 128) * D           # 2048 free elements per partition per slab
ROWS = BH_PER_CORE * 128        # 2048 dram rows per core, [ROWS, FREE] fp16
# slabs per chunk, tapered: small first chunk so compute starts early,
# small last chunk so the final store is short; big middle chunks amortize
# DVE per-instruction startup (measured 0.38 ns/elem at 8192-elem ops vs
# 0.49 at 4096)
CHUNK_PLAN = [1, 1, 2, 3, 3, 3, 2, 1]
assert sum(CHUNK_PLAN) == BH_PER_CORE
XIN_BUFS = 4
U_BUFS = 3
O_BUFS = 3
# fraction of each middle-chunk ADD's columns handed to GpSimd (Pool);
# DVE handles the rest. 0 disables the offload (GpSimd measured 2.5 ns/elem
# with ~700ns semaphore handling — it becomes the straggler if given work).
POOL_ADD_FRAC = 0.0

_prog_cache = {}


def _build_program():
    """Build (and cache) the SPMD Bass program for one core."""
    if "nc" in _prog_cache:
        return _prog_cache["nc"]

    import concourse.bacc as bacc
    import concourse.bass as bass
    import concourse.mybir as mybir
    import concourse.tile as tile

    f16 = mybir.dt.float16
    nc = bacc.Bacc(
        "TRN2", target_bir_lowering=False, debug=False, num_devices=N_CORES
    )
    x_d = nc.dram_tensor("x", [ROWS, FREE], f16, kind="ExternalInput").ap()
    ta_d = nc.dram_tensor("ta", [128, FREE], f16, kind="ExternalInput").ap()
    tb_d = nc.dram_tensor("tb", [128, FREE], f16, kind="ExternalInput").ap()
    o_d = nc.dram_tensor("out", [ROWS, FREE], f16, kind="ExternalOutput").ap()

    with tile.TileContext(nc) as tc:
        with (
            tc.tile_pool(name="tabs", bufs=1) as tabs,
            tc.tile_pool(name="xin", bufs=XIN_BUFS) as xin,
            tc.tile_pool(name="u", bufs=U_BUFS) as upool,
            tc.tile_pool(name="o", bufs=O_BUFS) as opool,
        ):
            # table loads go on the scalar ring (idle at start); pieces
            # ordered to unblock the first quarter-slab compute asap (which
            # needs tb+ta cols [0:qf)) while the sync ring pulls x
            tb = tabs.tile([128, FREE], f16)
            ta = tabs.tile([128, FREE], f16)
            qf = FREE // 4
            for piece in (
                (0, qf), (qf, 2 * qf), (2 * qf, 3 * qf), (3 * qf, FREE)
            ):
                lo, hi = piece
                nc.scalar.dma_start(tb[:, lo:hi], tb_d[:, lo:hi])
                nc.scalar.dma_start(ta[:, lo:hi], ta_d[:, lo:hi])

            def compute(xt, ut, ot, nsl, lo, sz):
                """u = x*tb; o = x*ta; o += pairswap(u) on cols [lo, lo+sz)
                of each of the nsl slabs (3-dim APs throughout: measured
                fastest on HW; 4-dim merged APs ran ~15% slower)."""
                if nsl == 1:
                    xs = xt[:, lo : lo + sz]
                    us = ut[:, lo : lo + sz]
                    os_ = ot[:, lo : lo + sz]
                    nc.vector.tensor_mul(us, xs, tb[:, lo : lo + sz])
                    nc.vector.tensor_mul(os_, xs, ta[:, lo : lo + sz])
                else:
                    assert lo == 0 and sz == nsl * FREE
                    x3 = xt[:].rearrange("p (j f) -> p j f", j=nsl)
                    u3 = ut[:].rearrange("p (j f) -> p j f", j=nsl)
                    o3 = ot[:].rearrange("p (j f) -> p j f", j=nsl)
                    ta_b = bass.AP(
                        ta[:].tensor, ta[:].offset,
                        [ta[:].ap[0], [0, nsl], ta[:].ap[1]],
                    )
                    tb_b = bass.AP(
                        tb[:].tensor, tb[:].offset,
                        [tb[:].ap[0], [0, nsl], tb[:].ap[1]],
                    )
                    nc.vector.tensor_mul(u3, x3, tb_b)
                    nc.vector.tensor_mul(o3, x3, ta_b)
                    us = ut[:]
                    os_ = ot[:]
                usw = us.rearrange("p (n two) -> p n two", two=2)[:, :, ::-1]
                os3 = os_.rearrange("p (n two) -> p n two", two=2)
                nc.vector.tensor_add(os3, os3, usw)

            row0 = 0
            for ci, nsl in enumerate(CHUNK_PLAN):
                first = ci == 0
                last = ci == len(CHUNK_PLAN) - 1
                cfree = nsl * FREE
                rows = x_d[row0 * 128 : (row0 + nsl) * 128, :]
                xt = xin.tile([128, cfree], f16, tag="xt")
                if first:
                    # quarter the first load so compute starts after 0.125 MiB
                    assert nsl == 1
                    q = cfree // 4
                    for qi in range(4):
                        nc.sync.dma_start(
                            xt[:, qi * q : (qi + 1) * q],
                            rows[:, qi * q : (qi + 1) * q],
                        )
                else:
                    src = rows.rearrange("(j p) f -> p j f", j=nsl)
                    nc.sync.dma_start(
                        xt[:].rearrange("p (j f) -> p j f", j=nsl), src
                    )

                ut = upool.tile([128, cfree], f16, tag="u")
                ot = opool.tile([128, cfree], f16, tag="o")
                orows = o_d[row0 * 128 : (row0 + nsl) * 128, :]

                if first:
                    # head chunk in quarters: compute can start after the
                    # first x quarter + table quarter land
                    q = cfree // 4
                    for qi in range(4):
                        compute(xt, ut, ot, 1, qi * q, q)
                        nc.scalar.dma_start(
                            orows[:, qi * q : (qi + 1) * q],
                            ot[:, qi * q : (qi + 1) * q],
                        )
                elif last:
                    # tail chunk in halves: overlap the final store with the
                    # second half's compute
                    h = cfree // 2
                    for hi in range(2):
                        compute(xt, ut, ot, 1, hi * h, h)
                        nc.scalar.dma_start(
                            orows[:, hi * h : (hi + 1) * h],
                            ot[:, hi * h : (hi + 1) * h],
                        )
                else:
                    compute(xt, ut, ot, nsl, 0, cfree)
                    dst = orows.rearrange("(j p) f -> p j f", j=nsl)
                    nc.scalar.dma_start(
                        dst, ot[:].rearrange("p (j f) -> p j f", j=nsl)
                    )
                row0 += nsl

    nc.compile()
    _prog_cache["nc"] = nc
    return nc


def _default_rotate_m(theta=10000.0):
    """Rebuild the reference's rotation buffer if the harness doesn't pass it."""
    half = D // 2
    try:  # replicate the reference's jax-f32 arithmetic exactly if possible
        import jax.numpy as jnp

        pos = np.asarray(jnp.arange(S, dtype=jnp.float32))
        inv_freq = np.asarray(
            theta ** (-(2.0 * jnp.arange(half, dtype=jnp.float32)) / D)
        )
        ang = np.asarray(pos[:, None] * inv_freq[None, :], dtype=np.float32)
        c, s = np.asarray(jnp.cos(ang)), np.asarray(jnp.sin(ang))
    except Exception:
        pos = np.arange(S, dtype=np.float32)
        exp = (-(2.0 * np.arange(half, dtype=np.float32)) / D).astype(np.float32)
        inv_freq = np.power(np.float32(theta), exp, dtype=np.float32)
        ang = (pos[:, None] * inv_freq[None, :]).astype(np.float32)
        c, s = np.cos(ang, dtype=np.float32), np.sin(ang, dtype=np.float32)
    idx = 2 * np.arange(half)
    r = np.zeros((S, D, D), dtype=np.float32)
    r[:, idx, idx] = c
    r[:, idx, idx + 1] = -s
    r[:, idx + 1, idx] = s
    r[:, idx + 1, idx + 1] = c
    return r


def _tables(token_positions, rotate_m):
    """Host-precompute the [128, FREE] fp16 A and B' tables.

    A[s,2k] = r[2k,2k], A[s,2k+1] = r[2k+1,2k+1]  (cos terms)
    B'[s,2k] = r[2k+1,2k], B'[s,2k+1] = r[2k,2k+1] (pre-pairswapped sin terms
    so that pairswap(x*B') lands the right products on the right lanes)."""
    if rotate_m is None:
        rotate_m = _default_rotate_m()
    r = np.asarray(rotate_m, dtype=np.float32)[np.asarray(token_positions)]
    idx = np.arange(D // 2) * 2
    a = r[:, idx, idx]            # x_even -> out_even
    b = r[:, idx, idx + 1]        # x_odd  -> out_even
    c = r[:, idx + 1, idx + 1]    # x_odd  -> out_odd
    d = r[:, idx + 1, idx]        # x_even -> out_odd
    A = np.empty((S, D), np.float32)
    A[:, 0::2] = a
    A[:, 1::2] = c
    Bp = np.empty((S, D), np.float32)
    Bp[:, 0::2] = d
    Bp[:, 1::2] = b
    return (
        np.ascontiguousarray(A.reshape(128, FREE)).astype(np.float16),
        np.ascontiguousarray(Bp.reshape(128, FREE)).astype(np.float16),
    )


def _in_maps(x, token_positions, rotate_m):
    ta, tb = _tables(token_positions, rotate_m)
    xs = np.asarray(x, dtype=np.float32).astype(np.float16).reshape(
        N_CORES, ROWS, FREE
    )
    xs = np.ascontiguousarray(xs)
    return [{"x": xs[i], "ta": ta, "tb": tb} for i in range(N_CORES)]


def _run(x, token_positions, rotate_m=None, trace=False, trace_cores=None):
    from concourse.bass_utils import run_bass_kernel_spmd

    nc = _build_program()
    in_maps = _in_maps(x, token_positions, rotate_m)
    res = run_bass_kernel_spmd(
        nc,
        in_maps,
        list(range(N_CORES)),
        trace=trace,
        trace_cores=trace_cores,
    )
    out = np.concatenate(
        [res.results[i]["out"].reshape(1, ROWS * FREE) for i in range(N_CORES)]
    ).reshape(B, H, S, D).astype(np.float32)
    return out, res


def kernel(x, token_positions, rotate_m=None, **_unused):
    out, _ = _run(x, token_positions, rotate_m, trace=False)
    return out


# revision 13
# speedup vs baseline: 1.0269x; 1.0269x over previous
"""MetaRoPE kernel for Trainium2, 8 NeuronCores — fp16 I/O, 2x-mode DVE,
merged muls, partial GpSimd offload.

Reference computation:
    r = rotate_m[token_positions]            # [S, D, D], block-diag 2x2 rotations
    out = einsum('bhsi,soi->bhso', x, r)     # x: [4, 32, 4096, 64] fp32

Because r is block-diagonal with 2x2 blocks, out = x * A + pairswap(x * B')
with host-precomputed tables A, B' of shape [S, D] (see _tables).

Precision/bandwidth: the harness gate is rel_err < 2e-2; fp16 end-to-end
(host converts x fp32->fp16, device computes in fp16, host converts the
fp16 result back) measures ~1.1e-3 and halves both HBM traffic and DVE
element cost. Plain InstTensorTensor ops hit the DVE 2x_1p perf mode with
packed fp16 (~0.49 ns/elem/partition measured, even with the stride -1
pair-swap operand); scalar_tensor_tensor would disable all perf modes.

Sharding: x reshaped to [128 (b,h) slabs, 4096, 64]; 16 slabs per core.
Each slab [4096*64] is viewed as [128 partitions, 2048 free] (contiguous per
partition; partition p holds positions 32p..32p+31). Tables are replicated
per core as one [128, 2*FREE] fp16 tile (tb | ta) matching that layout.

Per core the 16 slabs are processed in chunks (CHUNK_PLAN, tapered small at
the ends). Each chunk: one load (sync ring); ONE merged DVE multiply
computing u = x*tb and o = x*ta into one [128, 2*cfree] tile (x broadcast
via a step-0 AP dim, tables broadcast across slabs); one pair-swapped
in-place add o += pairswap(u) split by columns between DVE and GpSimd
(POOL_ADD_FRAC); one store (scalar ring). Steady state aims DMA-bound
(~17.9 MB/core at ~360 GB/s => ~50 us) with DVE (~38 us) + GpSimd (~16 us)
hidden underneath.
"""

import sys

import numpy as np

_TRN_REPO = "/opt/trn_rl_repo"
if _TRN_REPO not in sys.path:
    sys.path.insert(0, _TRN_REPO)

B, H, S, D = 4, 32, 4096, 64
BH = B * H                      # 128 (b,h) slabs
N_CORES = 8
BH_PER_CORE = BH // N_CORES     # 16 slabs per core
FREE = (S // 128) * D           # 2048 free elements per partition per slab
ROWS = BH_PER_CORE * 128        # 2048 dram rows per core, [ROWS, FREE] fp16
# slabs per chunk, tapered: small first chunk so compute starts early,
# small last chunk so the final store is short; big middle chunks amortize
# DVE per-instruction startup (measured 0.38 ns/elem at 8192-elem ops vs
# 0.49 at 4096)
CHUNK_PLAN = [1, 1, 2, 2, 2, 2, 2, 2, 1, 1]
assert sum(CHUNK_PLAN) == BH_PER_CORE
XIN_BUFS = 5
U_BUFS = 3
O_BUFS = 3
# fraction of each middle-chunk ADD's columns handed to GpSimd (Pool);
# DVE handles the rest. 0 disables the offload (GpSimd measured 2.5 ns/elem
# with ~700ns semaphore handling — it becomes the straggler if given work).
POOL_ADD_FRAC = 0.0

_prog_cache = {}


def _build_program():
    """Build (and cache) the SPMD Bass program for one core."""
    if "nc" in _prog_cache:
        return _prog_cache["nc"]

    import concourse.bacc as bacc
    import concourse.bass as bass
    import concourse.mybir as mybir
    import concourse.tile as tile

    f16 = mybir.dt.float16
    nc = bacc.Bacc(
        "TRN2", target_bir_lowering=False, debug=False, num_devices=N_CORES
    )
    x_d = nc.dram_tensor("x", [ROWS, FREE], f16, kind="ExternalInput").ap()
    ta_d = nc.dram_tensor("ta", [128, FREE], f16, kind="ExternalInput").ap()
    tb_d = nc.dram_tensor("tb", [128, FREE], f16, kind="ExternalInput").ap()
    o_d = nc.dram_tensor("out", [ROWS, FREE], f16, kind="ExternalOutput").ap()

    with tile.TileContext(nc) as tc:
        with (
            tc.tile_pool(name="tabs", bufs=1) as tabs,
            tc.tile_pool(name="xin", bufs=XIN_BUFS) as xin,
            tc.tile_pool(name="u", bufs=U_BUFS) as upool,
            tc.tile_pool(name="o", bufs=O_BUFS) as opool,
        ):
            # table loads go on the scalar ring (idle at start); pieces
            # ordered to unblock the first quarter-slab compute asap (which
            # needs tb+ta cols [0:qf)) while the sync ring pulls x
            tb = tabs.tile([128, FREE], f16)
            ta = tabs.tile([128, FREE], f16)
            hf = FREE // 2
            nc.scalar.dma_start(tb[:, :hf], tb_d[:, :hf])
            nc.scalar.dma_start(ta[:, :hf], ta_d[:, :hf])
            nc.scalar.dma_start(tb[:, hf:], tb_d[:, hf:])
            nc.scalar.dma_start(ta[:, hf:], ta_d[:, hf:])

            def compute(xt, ut, ot, nsl, lo, sz):
                """u = x*tb; o = x*ta; o += pairswap(u) on cols [lo, lo+sz)
                of each of the nsl slabs (3-dim APs throughout: measured
                fastest on HW; 4-dim merged APs ran ~15% slower)."""
                if nsl == 1:
                    xs = xt[:, lo : lo + sz]
                    us = ut[:, lo : lo + sz]
                    os_ = ot[:, lo : lo + sz]
                    nc.vector.tensor_mul(us, xs, tb[:, lo : lo + sz])
                    nc.vector.tensor_mul(os_, xs, ta[:, lo : lo + sz])
                else:
                    assert lo == 0 and sz == nsl * FREE
                    x3 = xt[:].rearrange("p (j f) -> p j f", j=nsl)
                    u3 = ut[:].rearrange("p (j f) -> p j f", j=nsl)
                    o3 = ot[:].rearrange("p (j f) -> p j f", j=nsl)
                    ta_b = bass.AP(
                        ta[:].tensor, ta[:].offset,
                        [ta[:].ap[0], [0, nsl], ta[:].ap[1]],
                    )
                    tb_b = bass.AP(
                        tb[:].tensor, tb[:].offset,
                        [tb[:].ap[0], [0, nsl], tb[:].ap[1]],
                    )
                    nc.vector.tensor_mul(u3, x3, tb_b)
                    nc.vector.tensor_mul(o3, x3, ta_b)
                    us = ut[:]
                    os_ = ot[:]
                usw = us.rearrange("p (n two) -> p n two", two=2)[:, :, ::-1]
                os3 = os_.rearrange("p (n two) -> p n two", two=2)
                nc.vector.tensor_add(os3, os3, usw)

            row0 = 0
            for ci, nsl in enumerate(CHUNK_PLAN):
                first = ci == 0
                last = ci == len(CHUNK_PLAN) - 1
                cfree = nsl * FREE
                rows = x_d[row0 * 128 : (row0 + nsl) * 128, :]
                xt = xin.tile([128, cfree], f16, tag="xt")
                if first:
                    # split the first load so compute can start after 0.25 MiB
                    assert nsl == 1
                    h = cfree // 2
                    nc.sync.dma_start(xt[:, :h], rows[:, :h])
                    nc.sync.dma_start(xt[:, h:], rows[:, h:])
                else:
                    src = rows.rearrange("(j p) f -> p j f", j=nsl)
                    nc.sync.dma_start(
                        xt[:].rearrange("p (j f) -> p j f", j=nsl), src
                    )

                ut = upool.tile([128, cfree], f16, tag="u")
                ot = opool.tile([128, cfree], f16, tag="o")
                orows = o_d[row0 * 128 : (row0 + nsl) * 128, :]

                if first or last:
                    # head/tail chunk in halves: head starts computing after
                    # the first half-load; tail overlaps the final store with
                    # the second half's compute
                    h = cfree // 2
                    for hi in range(2):
                        compute(xt, ut, ot, 1, hi * h, h)
                        nc.scalar.dma_start(
                            orows[:, hi * h : (hi + 1) * h],
                            ot[:, hi * h : (hi + 1) * h],
                        )
                else:
                    compute(xt, ut, ot, nsl, 0, cfree)
                    dst = orows.rearrange("(j p) f -> p j f", j=nsl)
                    nc.scalar.dma_start(
                        dst, ot[:].rearrange("p (j f) -> p j f", j=nsl)
                    )
                row0 += nsl

    nc.compile()
    _prog_cache["nc"] = nc
    return nc


def _default_rotate_m(theta=10000.0):
    """Rebuild the reference's rotation buffer if the harness doesn't pass it."""
    half = D // 2
    try:  # replicate the reference's jax-f32 arithmetic exactly if possible
        import jax.numpy as jnp

        pos = np.asarray(jnp.arange(S, dtype=jnp.float32))
        inv_freq = np.asarray(
            theta ** (-(2.0 * jnp.arange(half, dtype=jnp.float32)) / D)
        )
        ang = np.asarray(pos[:, None] * inv_freq[None, :], dtype=np.float32)
        c, s = np.asarray(jnp.cos(ang)), np.asarray(jnp.sin(ang))
    except Exception:
        pos = np.arange(S, dtype=np.float32)
        exp = (-(2.0 * np.arange(half, dtype=np.float32)) / D).astype(np.float32)
        inv_freq = np.power(np.float32(theta), exp, dtype=np.float32)
        ang = (pos[:, None] * inv_freq[None, :]).astype(np.float32)
        c, s = np.cos(ang, dtype=np.float32), np.sin(ang, dtype=np.float32)
    idx = 2 * np.arange(half)
    r = np.zeros((S, D, D), dtype=np.float32)
    r[:, idx, idx] = c
    r[:, idx, idx + 1] = -s
    r[:, idx + 1, idx] = s
    r[:, idx + 1, idx + 1] = c
    return r


def _tables(token_positions, rotate_m):
    """Host-precompute the [128, FREE] fp16 A and B' tables.

    A[s,2k] = r[2k,2k], A[s,2k+1] = r[2k+1,2k+1]  (cos terms)
    B'[s,2k] = r[2k+1,2k], B'[s,2k+1] = r[2k,2k+1] (pre-pairswapped sin terms
    so that pairswap(x*B') lands the right products on the right lanes)."""
    if rotate_m is None:
        rotate_m = _default_rotate_m()
    r = np.asarray(rotate_m, dtype=np.float32)[np.asarray(token_positions)]
    idx = np.arange(D // 2) * 2
    a = r[:, idx, idx]            # x_even -> out_even
    b = r[:, idx, idx + 1]        # x_odd  -> out_even
    c = r[:, idx + 1, idx + 1]    # x_odd  -> out_odd
    d = r[:, idx + 1, idx]        # x_even -> out_odd
    A = np.empty((S, D), np.float32)
    A[:, 0::2] = a
    A[:, 1::2] = c
    Bp = np.empty((S, D), np.float32)
    Bp[:, 0::2] = d
    Bp[:, 1::2] = b
    return (
        np.ascontiguousarray(A.reshape(128, FREE)).astype(np.float16),
        np.ascontiguousarray(Bp.reshape(128, FREE)).astype(np.float16),
    )


def _in_maps(x, token_positions, rotate_m):
    ta, tb = _tables(token_positions, rotate_m)
    xs = np.asarray(x, dtype=np.float32).astype(np.float16).reshape(
        N_CORES, ROWS, FREE
    )
    xs = np.ascontiguousarray(xs)
    return [{"x": xs[i], "ta": ta, "tb": tb} for i in range(N_CORES)]


def _run(x, token_positions, rotate_m=None, trace=False, trace_cores=None):
    from concourse.bass_utils import run_bass_kernel_spmd

    nc = _build_program()
    in_maps = _in_maps(x, token_positions, rotate_m)
    res = run_bass_kernel_spmd(
        nc,
        in_maps,
        list(range(N_CORES)),
        trace=trace,
        trace_cores=trace_cores,
    )
    out = np.concatenate(
        [res.results[i]["out"].reshape(1, ROWS * FREE) for i in range(N_CORES)]
    ).reshape(B, H, S, D).astype(np.float32)
    return out, res


def kernel(x, token_positions, rotate_m=None, **_unused):
    out, _ = _run(x, token_positions, rotate_m, trace=False)
    return out
